# revision 38
# baseline (speedup 1.0000x reference)
"""
MultiHeadLatentMoE layer as a Bass/Tile kernel for 8 Trainium2 NeuronCores.

Problem (T=8192, D=1024, NH=8 heads, DH=128, NE=8 experts/head, top-2, DHID=512):
    h      = (x @ in_w.T + in_b).reshape(T, NH, DH)
    logits = einsum('tnd,ned->tne', h, router_w)            (fp32)
    gate   = scatter(softmax(top2(logits)))                  (T, NH, NE)
    hid    = gelu(einsum('tnd,nefd->tnef', h, w_in))         (exact erf gelu)
    ye     = einsum('tnef,nefd->tned', hid, w_out)
    y      = einsum('tne,tned->tnd', gate, ye)
    out    = y.reshape(T, NH*DH) @ out_w.T + out_b

Sharding: data-parallel over tokens (1024 tokens/core, all heads+experts
local) -> zero collectives.  Per-core output shard is (D, T_loc) transposed;
host concatenates.

This version dispatches experts SPARSELY (top-2 of 8): per (head, expert)
token lists are built on-device (PE prefix-sum matmuls + gpsimd
sparse_gather), tokens are gathered with SBUF-source dma_gather into a
capacity-384 workspace, expert FFNs run in bf16 on gathered tokens only
(4x fewer MACs than dense), results are gathered back per token and
gate-combined.  Routing decisions are bit-exact vs the fp32 reference:
logits come from x @ R (R = in_w^T-blocks @ router_w folded on host in
fp64) with a 3-term fp22 hi/lo split (verified 0/65536 top-2 flips on the
reference input; min top2-vs-3rd logit gap 4.1e-7 vs split error ~1e-9).

Capacity C=384 per (head, expert): reference input maxes at 312 tokens.
"""

import sys

for _p in ("/opt/trn_rl_repo", "/root/.axon_site/_ro/trn_rl_repo"):
    if _p not in sys.path:
        sys.path.append(_p)

import numpy as np
import ml_dtypes

import concourse.bass as bass
import concourse.mybir as mybir
import concourse.tile as tile
from concourse import bacc
from concourse.bass_utils import run_bass_kernel_spmd
from concourse.masks import make_identity

T, D, NH, DH, NE, TOPK, DHID = 8192, 1024, 8, 128, 8, 2, 512
NCORES = 8
TLOC = T // NCORES          # 1024 tokens per core
P = 128
KT = D // P                 # 8 contraction k-tiles for D=1024
NT = TLOC // P              # 8 token tiles of 128
FT = DHID // P              # 4 f-tiles per expert
C = 384                     # per-(head,expert) token capacity (max seen: 312)
CR = C // P                 # 3 capacity chunks of 128
NEC = NE * C                # 3072 workspace slots per head
F32 = mybir.dt.float32
F32R = mybir.dt.float32r
BF16 = mybir.dt.bfloat16
I16 = mybir.dt.int16
U32 = mybir.dt.uint32

_CACHED = None
TRACE = False          # set True (e.g. from test.py) to neuron-profile the run
LAST_RESULT = None     # BassKernelResults of the most recent kernel() call

import os as _os
_PHASES = int(_os.environ.get("KPHASES", "5"))  # debug: 1=R 2=+D 3=+H 4=+E 5=all


def build_program():
    nc = bacc.Bacc()

    # ---- DRAM parameters ----
    xt_hi = nc.dram_tensor("xt_hi", [D, TLOC], F32R, kind="ExternalInput")
    xt_lo = nc.dram_tensor("xt_lo", [D, TLOC], F32R, kind="ExternalInput")
    inwt = nc.dram_tensor("inwt", [D, D], F32R, kind="ExternalInput")
    r_hi = nc.dram_tensor("r_hi", [D, NH * NE], F32R, kind="ExternalInput")
    r_lo = nc.dram_tensor("r_lo", [D, NH * NE], F32R, kind="ExternalInput")
    in_b = nc.dram_tensor("in_b", [D], F32, kind="ExternalInput")
    out_b = nc.dram_tensor("out_b", [D], F32, kind="ExternalInput")
    # packed expert weights per (n, e): [:, :DHID] = w_in^T (dh, f),
    # [:, DHID:] = w_out packed (f%128, ft, dh)
    wio = nc.dram_tensor("wio", [NH, NE, P, 2 * DHID], BF16, kind="ExternalInput")
    out_wt = nc.dram_tensor("out_wt", [D, D], BF16, kind="ExternalInput")
    # host consts.  Token ORD for list/workspace ordering is sigma(t) =
    # a*128 + k*16 + b where t = 128k + 8b + a (p = 8b + a): this makes the
    # 128->16 partition folds expressible as single strided DMAs.
    l_intra = nc.dram_tensor("l_intra", [P, P], F32, kind="ExternalInput")
    lstrict64 = nc.dram_tensor("lstrict64", [64, 64], F32, kind="ExternalInput")
    seltot = nc.dram_tensor("seltot", [NT, P, 64], F32, kind="ExternalInput")
    blkbc = nc.dram_tensor("blkbc", [NT, 64, P], F32, kind="ExternalInput")
    iota_tp1 = nc.dram_tensor("iota_tp1", [P, NT], F32, kind="ExternalInput")
    iota_ec = nc.dram_tensor("iota_ec", [P, NE], F32, kind="ExternalInput")

    # DRAM scratch for partition-regrouping bounces
    tok_dram = nc.dram_tensor("tok_dram", [16, NH, NE, C // 16], I16)
    pos_dram = nc.dram_tensor("pos_dram", [NH, TOPK, 16, 64], I16)
    w_dram = nc.dram_tensor("w_dram", [TOPK, NH, TLOC], BF16)

    out_t = nc.dram_tensor("out_t", [D, TLOC], F32, kind="ExternalOutput")

    Act = mybir.ActivationFunctionType
    Alu = mybir.AluOpType
    AxX = mybir.AxisListType.X

    with tile.TileContext(nc) as tc:
        with (
            tc.tile_pool(name="persist", bufs=1) as persist,
            tc.tile_pool(name="work", bufs=2) as work,
        ):
            # ---- persistent SBUF ----
            ident = persist.tile([P, P], F32, tag="ident")
            make_identity(nc, ident)
            h_tok = persist.tile([P, NT, NH, DH], BF16, tag="h_tok")  # token-major
            in_bc = persist.tile([P, D], F32, tag="in_bc")
            y_sb = persist.tile([P, NH, TLOC], BF16, tag="y")
            outb_sb = persist.tile([P, KT], F32, tag="outb")
            lint_sb = persist.tile([P, P], F32, tag="lint")
            lst64_sb = persist.tile([64, 64], F32, tag="lst64")
            seltot_sb = persist.tile([P, NT, 64], F32, tag="seltot")
            blkbc_sb = persist.tile([64, NT, P], F32, tag="blkbc")
            iot_sb = persist.tile([P, NT], F32, tag="iot")      # t+1 per tile
            ioe_sb = persist.tile([P, NE], F32, tag="ioe")      # e*C
            # routing state
            val_all = persist.tile([P, NH, NT, NE], F32, tag="val")
            eq1_all = persist.tile([P, NT, NH, NE], F32, tag="eq1")
            eq2_all = persist.tile([P, NT, NH, NE], F32, tag="eq2")
            w_all = persist.tile([P, TOPK, NH, NT], BF16, tag="w_all")
            pos_all = persist.tile([P, TOPK, NH, NT], F32, tag="pos")
            pos16_all = persist.tile([P, TOPK, NH, NT], I16, tag="pos16")
            sgin = persist.tile([16, NH, NE, TLOC // 16], F32, tag="sgin")
            sgout = persist.tile([16, NH, NE, C // 16], F32, tag="sgout")
            nf_sb = persist.tile([1, NH * NE], U32, tag="nf")
            tokrep = persist.tile([P, NH, NE, C // 16], I16, tag="tokrep")
            posrep = persist.tile([P, NH, TOPK, 64], I16, tag="posrep")

            nc.sync.dma_start(in_bc[:], in_b[:][None, :].to_broadcast([P, D]))
            nc.sync.dma_start(outb_sb[:], out_b[:].rearrange("(m p) -> p m", p=P))
            nc.sync.dma_start(lint_sb[:], l_intra[:])
            nc.sync.dma_start(lst64_sb[:], lstrict64[:])
            nc.sync.dma_start(
                seltot_sb[:], seltot[:].rearrange("k p j -> p k j"))
            nc.sync.dma_start(
                blkbc_sb[:], blkbc[:].rearrange("k j p -> j k p"))
            nc.sync.dma_start(iot_sb[:], iota_tp1[:])
            nc.sync.dma_start(ioe_sb[:], iota_ec[:])

            with tc.tile_pool(name="xpool", bufs=1) as xpool:
                x_hi = xpool.tile([P, KT, TLOC], F32R, tag="x_hi")
                x_lo = xpool.tile([P, KT, TLOC], F32R, tag="x_lo")
                inwt_sb = xpool.tile([P, KT, D], F32R, tag="inwt")
                rhi_sb = xpool.tile([P, KT, NH * NE], F32R, tag="rhi")
                rlo_sb = xpool.tile([P, KT, NH * NE], F32R, tag="rlo")
                for kt in range(KT):
                    sl = slice(kt * P, (kt + 1) * P)
                    nc.sync.dma_start(x_hi[:, kt, :], xt_hi[sl, :])
                    nc.sync.dma_start(x_lo[:, kt, :], xt_lo[sl, :])
                    nc.sync.dma_start(inwt_sb[:, kt, :], inwt[sl, :])
                nc.sync.dma_start(
                    rhi_sb[:], r_hi[:].rearrange("(kt p) f -> p kt f", p=P))
                nc.sync.dma_start(
                    rlo_sb[:], r_lo[:].rearrange("(kt p) f -> p kt f", p=P))

                # ============ Phase R: router logits^T + top-2 routing =======
                with tc.tile_pool(name="rpsum", bufs=1, space="PSUM") as rpsum:
                    lg_sb = work.tile([64, TLOC], F32, tag="lgT", bufs=1)
                    for tt in range(2):
                        tsl = slice(tt * 512, (tt + 1) * 512)
                        lgt_ps = rpsum.tile([64, 512], F32, tag="lgt", bufs=2)
                        terms = [(rhi_sb, x_hi), (rhi_sb, x_lo), (rlo_sb, x_hi)]
                        for i, (rv, xv) in enumerate(terms):
                            for kt in range(KT):
                                nc.tensor.matmul(
                                    lgt_ps[:],
                                    lhsT=rv[:, kt, :],
                                    rhs=xv[:, kt, tsl],
                                    start=(i == 0 and kt == 0),
                                    stop=(i == 2 and kt == KT - 1),
                                )
                        nc.scalar.copy(lg_sb[:, tsl], lgt_ps[:])

                    # per-tile top-2 + encode
                    rank_ps = rpsum.tile([P, NT, NH, NE], F32, tag="rank")
                    tot_ps = rpsum.tile([64, NH * NE], F32, tag="tot")
                    for k in range(NT):
                        ksl = slice(k * P, (k + 1) * P)
                        tp_ps = rpsum.tile([P, 64], F32, tag="tp", bufs=2)
                        nc.tensor.transpose(
                            tp_ps[:], lg_sb[:, ksl], ident[:64, :64])
                        lg = work.tile([P, NH, NE], F32, tag="lg")
                        nc.vector.tensor_copy(
                            lg[:].rearrange("p n e -> p (n e)"), tp_ps[:])
                        m1 = work.tile([P, NH], F32, tag="m1")
                        nc.vector.tensor_reduce(m1[:], lg[:], AxX, Alu.max)
                        eq1 = eq1_all[:, k, :, :]
                        nc.vector.tensor_tensor(
                            eq1, lg[:], m1[:, :, None].to_broadcast([P, NH, NE]),
                            Alu.is_equal)
                        msk = work.tile([P, NH, NE], F32, tag="msk")
                        nc.vector.scalar_tensor_tensor(
                            msk[:], eq1, -1e30, lg[:], Alu.mult, Alu.add)
                        m2 = work.tile([P, NH], F32, tag="m2")
                        nc.vector.tensor_reduce(m2[:], msk[:], AxX, Alu.max)
                        eq2 = eq2_all[:, k, :, :]
                        nc.vector.tensor_tensor(
                            eq2, lg[:], m2[:, :, None].to_broadcast([P, NH, NE]),
                            Alu.is_equal)
                        dm = work.tile([P, NH], F32, tag="dm")
                        nc.vector.tensor_sub(dm[:], m2[:], m1[:])
                        # w2 = sigmoid(m2-m1); w1 = 1-w2  (bf16 gates)
                        nc.scalar.activation(
                            w_all[:, 1, :, k], dm[:], Act.Sigmoid)
                        nc.vector.tensor_scalar(
                            w_all[:, 0, :, k], w_all[:, 1, :, k],
                            -1.0, 1.0, Alu.mult, Alu.add)
                        m = work.tile([P, NH, NE], F32, tag="m")
                        nc.vector.tensor_tensor(m[:], eq1, eq2, Alu.add)
                        # val = m*(t+1) - 1   (masked-out -> -1, else token id)
                        tmp = work.tile([P, NH, NE], F32, tag="tmp")
                        nc.vector.tensor_scalar(
                            tmp[:], m[:], iot_sb[:, k:k + 1], None, Alu.mult)
                        nc.vector.tensor_scalar(
                            val_all[:, :, k, :], tmp[:], -1.0, None, Alu.add)
                        # intra-group exclusive prefix (over b within (a, k))
                        # + block totals (block j = a*8 + k, a = p%8)
                        for n in range(NH):
                            nc.tensor.matmul(
                                rank_ps[:, k, n, :],
                                lhsT=lint_sb[:],
                                rhs=m[:, n, :],
                                start=(k == 0 and n == 0), stop=False,
                                skip_group_check=True,
                            )
                        nc.tensor.matmul(
                            tot_ps[:],
                            lhsT=seltot_sb[:, k, :],
                            rhs=m[:].rearrange("p n e -> p (n e)"),
                            start=(k == 0), stop=(k == NT - 1),
                            skip_group_check=True,
                        )
                    # exclusive prefix over the 64 (a, k) blocks
                    tot_sb = work.tile([64, NH * NE], F32, tag="tot_sb", bufs=1)
                    nc.vector.tensor_copy(tot_sb[:], tot_ps[:])
                    cum_ps = rpsum.tile([64, NH * NE], F32, tag="cum")
                    nc.tensor.matmul(
                        cum_ps[:], lhsT=lst64_sb[:], rhs=tot_sb[:],
                        start=True, stop=True, skip_group_check=True)
                    cum_sb = work.tile([64, NH * NE], F32, tag="cum_sb", bufs=1)
                    nc.vector.tensor_copy(cum_sb[:], cum_ps[:])
                    for k in range(NT):
                        # add block-cum of block (p%8, k) to each token's rank
                        nc.tensor.matmul(
                            rank_ps[:, k, :, :].rearrange("p n e -> p (n e)"),
                            lhsT=blkbc_sb[:, k, :],
                            rhs=cum_sb[:],
                            start=False, stop=True,
                            skip_group_check=True,
                        )
                    # pos_s[t] = sum_e eq_s * (e*C + rank)
                    for k in range(NT):
                        tmp = work.tile([P, NH, NE], F32, tag="tmp")
                        nc.vector.tensor_tensor(
                            tmp[:], rank_ps[:, k, :, :],
                            ioe_sb[:, None, :].to_broadcast([P, NH, NE]),
                            Alu.add)
                        for s, eqs in ((0, eq1_all), (1, eq2_all)):
                            pr = work.tile([P, NH, NE], F32, tag="pr")
                            nc.vector.tensor_tensor(
                                pr[:], eqs[:, k, :, :], tmp[:], Alu.mult)
                            nc.vector.tensor_reduce(
                                pos_all[:, s, :, k], pr[:], AxX, Alu.add)
                    nc.vector.tensor_scalar_min(
                        pos_all[:], pos_all[:], float(NEC - 1))
                    nc.vector.tensor_copy(pos16_all[:], pos_all[:])

                # ============ Phase D: token-list bookkeeping ================
                # regroup 128 partitions -> 16-partition wrap, sigma order:
                # sgin[b, (a k)] = val_all[8b+a, n, k, e]
                dma_eng = [nc.sync, nc.scalar]
                for n in range(NH):
                    for e in range(NE):
                        dma_eng[(n * NE + e) % 2].dma_start(
                            sgin[:, n, e, :].rearrange(
                                "b (a k) -> b a k", a=8),
                            val_all[:, n, :, e])
                nc.vector.memset(sgout[:], 0.0)
                for n in range(NH):
                    for e in range(NE):
                        nc.gpsimd.sparse_gather(
                            sgout[:, n, e, :], sgin[:, n, e, :],
                            num_found=nf_sb[:, n * NE + e:n * NE + e + 1])
                # decode: clamp tail (-1) to token 0; values are exact ints
                a = work.tile([16, NH, NE, C // 16], F32, tag="a", bufs=1)
                nc.vector.tensor_scalar_max(a[:], sgout[:], 0.0)
                nc.vector.tensor_scalar_min(a[:], a[:], float(TLOC - 1))
                tok16 = work.tile([16, NH, NE, C // 16], I16, tag="tok16",
                                  bufs=1)
                nc.vector.tensor_copy(tok16[:], a[:])
                nc.sync.dma_start(tok_dram[:], tok16[:])
                # gates to DRAM in sigma order for per-token broadcast reads
                for s in range(TOPK):
                    for n in range(NH):
                        nc.scalar.dma_start(
                            w_dram[s, n].rearrange("(a k b) -> b a k",
                                                   a=8, k=NT),
                            w_all[:, s, n, :])
                # pos idx lists to DRAM (16-wrap, sigma order)
                for n in range(NH):
                    for s in range(TOPK):
                        nc.scalar.dma_start(
                            pos_dram[n, s].rearrange("b (a k) -> b a k", a=8),
                            pos16_all[:, s, n, :])
                # replicate idx lists to all 128 partitions
                for q in range(8):
                    qsl = slice(16 * q, 16 * (q + 1))
                    nc.sync.dma_start(tokrep[qsl], tok_dram[:])
                    for s in range(TOPK):
                        nc.sync.dma_start(
                            posrep[qsl, :, s, :],
                            pos_dram[:, s].rearrange("n b f -> b n f"))

                # ============ Phase H: in-projection (token-major) ===========
                with tc.tile_pool(name="hpsum", bufs=1, space="PSUM") as hpsum:
                    for half in range(2):
                        fsl = slice(half * 512, (half + 1) * 512)
                        for c in range(NT):
                            csl = slice(c * P, (c + 1) * P)
                            hp = hpsum.tile([P, 512], F32, tag="hp", bufs=4)
                            for kt in range(KT):
                                nc.tensor.matmul(
                                    hp[:],
                                    lhsT=x_hi[:, kt, csl],
                                    rhs=inwt_sb[:, kt, fsl],
                                    start=(kt == 0),
                                    stop=(kt == KT - 1),
                                )
                            nc.vector.tensor_tensor(
                                h_tok[:, c, half * 4:(half + 1) * 4, :]
                                .rearrange("p n d -> p (n d)"),
                                hp[:], in_bc[:, fsl], Alu.add)

            # ============ Phase E: sparse experts ========================
            with (
                tc.tile_pool(name="wpool", bufs=2) as wpool,
                tc.tile_pool(name="hgpool", bufs=2) as hgpool,
                tc.tile_pool(name="wspool", bufs=2) as wspool,
                tc.tile_pool(name="gpool", bufs=3) as gpool,
                tc.tile_pool(name="ypool", bufs=2) as ypool,
                tc.tile_pool(name="epsum", bufs=1, space="PSUM") as epsum,
            ):
                hg_tiles = {}

                def issue_hgather(n):
                    # SWDGE dma_gather wedges above ~768 idxs -> chunk by 768
                    hg = hgpool.tile([P, 1, NEC], BF16, tag="hg")
                    tokv = tokrep[:, n, :, :].rearrange("p e f -> p (e f)")
                    for c0 in range(0, NEC, 768):
                        nc.gpsimd.dma_gather(
                            hg[:, :, c0:c0 + 768], h_tok[:],
                            tokv[:, c0 // 16:(c0 + 768) // 16],
                            num_idxs=768, num_idxs_reg=768,
                            elem_size=DH, transpose=True,
                            sbuf_tokens_per_rank=P,
                            sbuf_free_dim_per_rank=NH * DH * 2,
                            sbuf_byte_offset=n * DH * 2,
                        )
                    hg_tiles[n] = hg

                issue_hgather(0)
                for n in range(NH):
                    wio_sb = wpool.tile([P, NE, 2 * DHID], BF16, tag="wio")
                    nc.sync.dma_start(
                        wio_sb[:], wio[n].rearrange("e p f -> p e f"))
                    gb1 = gpool.tile([P, TLOC], BF16, tag="gb1")
                    gb2 = gpool.tile([P, TLOC], BF16, tag="gb2")
                    nc.sync.dma_start(
                        gb1[:], w_dram[0, n][None, :].to_broadcast([P, TLOC]))
                    nc.sync.dma_start(
                        gb2[:], w_dram[1, n][None, :].to_broadcast([P, TLOC]))
                    ws = wspool.tile([P, NE * CR, DH], BF16, tag="ws")
                    hg = hg_tiles.pop(n)
                    for e in range(NE):
                        esl = slice(e * C, (e + 1) * C)
                        hid_a = epsum.tile([P, 2, 512], F32, tag="hid_a",
                                           bufs=1)
                        hid_b = epsum.tile([P, 2, 512], F32, tag="hid_b",
                                           bufs=1)
                        hidg = gpool.tile([P, FT, C], BF16, tag="hidg")
                        for hf, hps in ((0, hid_a), (1, hid_b)):
                            for i in range(2):
                                f = hf * 2 + i
                                nc.tensor.matmul(
                                    hps[:, i, :C],
                                    lhsT=wio_sb[:, e, f * P:(f + 1) * P],
                                    rhs=hg[:, 0, esl],
                                    start=True, stop=True,
                                )
                            nc.scalar.activation(
                                hidg[:, hf * 2:(hf + 1) * 2, :],
                                hps[:, :, :C], Act.Gelu)
                        ye_ps = epsum.tile([P, CR, DH], F32, tag="ye", bufs=2)
                        for tc_ in range(CR):
                            csl = slice(tc_ * P, (tc_ + 1) * P)
                            for f in range(FT):
                                nc.tensor.matmul(
                                    ye_ps[:, tc_, :],
                                    lhsT=hidg[:, f, csl],
                                    rhs=wio_sb[:, e,
                                               DHID + f * P:DHID + (f + 1) * P],
                                    start=(tc_ == 0 and f == 0),
                                    stop=(tc_ == CR - 1 and f == FT - 1),
                                    skip_group_check=True,
                                )
                        nc.vector.tensor_copy(
                            ws[:, e * CR:(e + 1) * CR, :], ye_ps[:])
                    if n + 1 < NH:
                        issue_hgather(n + 1)
                    # gather-back both slots, then gate-combine per token
                    yb = ypool.tile([P, 1, TOPK * TLOC], BF16, tag="yb")
                    posv = posrep[:, n, :, :].rearrange("p s f -> p (s f)")
                    for c0 in range(0, TOPK * TLOC, 512):
                        nc.gpsimd.dma_gather(
                            yb[:, :, c0:c0 + 512], ws[:],
                            posv[:, c0 // 16:(c0 + 512) // 16],
                            num_idxs=512, num_idxs_reg=512,
                            elem_size=DH, transpose=True,
                            sbuf_tokens_per_rank=P,
                            sbuf_free_dim_per_rank=DH * 2,
                            sbuf_byte_offset=0,
                        )
                    y1g = ypool.tile([P, TLOC], BF16, tag="y1g")
                    nc.vector.tensor_tensor(
                        y1g[:], yb[:, 0, :TLOC], gb1[:], Alu.mult)
                    y2g = ypool.tile([P, TLOC], BF16, tag="y2g")
                    nc.vector.tensor_tensor(
                        y2g[:], yb[:, 0, TLOC:], gb2[:], Alu.mult)
                    nc.vector.tensor_tensor(
                        y_sb[:, n, :], y1g[:], y2g[:], Alu.add)

            # ============ Phase O: out-projection ========================
            with (
                tc.tile_pool(name="opool", bufs=2) as opool,
                tc.tile_pool(name="opsum", bufs=1, space="PSUM") as opsum,
            ):
                ow_sb = opool.tile([P, KT, D], BF16, tag="ow", bufs=1)
                nc.sync.dma_start(
                    ow_sb[:], out_wt[:].rearrange("(kt p) d -> p kt d", p=P))
                for mt in range(KT):
                    msl = slice(mt * P, (mt + 1) * P)
                    o_sb = opool.tile([P, TLOC], F32, tag="osb")
                    for tt in range(2):
                        tsl = slice(tt * 512, (tt + 1) * 512)
                        o_ps = opsum.tile([P, 512], F32, tag="ops", bufs=4)
                        for kt in range(KT):
                            nc.tensor.matmul(
                                o_ps[:],
                                lhsT=ow_sb[:, kt, msl],
                                rhs=y_sb[:, kt, tsl],
                                start=(kt == 0),
                                stop=(kt == KT - 1),
                            )
                        nc.scalar.activation(
                            o_sb[:, tsl], o_ps[:], Act.Identity,
                            bias=outb_sb[:, mt:mt + 1])
                    nc.sync.dma_start(out_t[msl, :], o_sb[:])

    nc.compile()
    return nc


def _trunc22(a):
    """FP32 -> FP22 truncation (the read path of float32r matmuls)."""
    return (np.ascontiguousarray(a, np.float32).view(np.uint32)
            & np.uint32(0xFFFFE000)).view(np.float32)


def _bf16(a):
    return np.ascontiguousarray(a, np.float32).astype(ml_dtypes.bfloat16)


def _prep(x, in_w, in_b, router_w, w_in, w_out, out_w, out_b):
    """Host-side layout prep; returns per-core in_maps."""
    x = np.ascontiguousarray(x, dtype=np.float32)
    in_wt = np.ascontiguousarray(in_w.T, dtype=np.float32)           # (D, D)
    # R[d, (n,e)] = sum_dh in_w^T[d, (n,dh)] router_w[n, e, dh]  (fp64 exact)
    R = np.einsum(
        'dnh,neh->dne',
        in_wt.astype(np.float64).reshape(D, NH, DH),
        np.asarray(router_w, np.float64)).astype(np.float32).reshape(D, NH * NE)
    R_hi = _trunc22(R)
    R_lo = _trunc22(R - R_hi)
    # in_b enters logits as rb[n,e] = sum_dh in_b[(n,dh)] router_w[n,e,dh];
    # zero for this problem (asserted) — nonzero would need a logit bias add.
    rb = np.einsum('nh,neh->ne', np.asarray(in_b, np.float64).reshape(NH, DH),
                   np.asarray(router_w, np.float64))
    assert np.abs(rb).max() < 1e-30, "nonzero in_b needs router bias support"

    # packed expert weights
    wi_t = np.ascontiguousarray(
        np.asarray(w_in, np.float32).transpose(0, 1, 3, 2))  # (NH,NE,DH,DHID)
    wo_p = (np.asarray(w_out, np.float32)
            .reshape(NH, NE, FT, P, DH).transpose(0, 1, 3, 2, 4)
            .reshape(NH, NE, P, DHID))                       # (NH,NE,128,512)
    wio = np.concatenate([wi_t, wo_p], axis=-1)              # (NH,NE,128,1024)

    # consts (sigma token order: block j = (p%8)*8 + k; within-block order
    # is b = p//8)
    pi = np.arange(P)
    # l_intra[p', p] = 1 iff same a-group (p'%8 == p%8) and p'//8 < p//8
    l_intra = ((pi[:, None] % 8 == pi[None, :] % 8)
               & (pi[:, None] // 8 < pi[None, :] // 8)).astype(np.float32)
    j64 = np.arange(64)
    lstrict64 = (j64[:, None] < j64[None, :]).astype(np.float32)
    # seltot[k][p, j] = blkbc[k][j, p] = 1 iff j == (p%8)*8 + k
    sel = np.zeros((NT, P, 64), np.float32)
    for k in range(NT):
        sel[k, pi, (pi % 8) * 8 + k] = 1.0
    blkbc = np.ascontiguousarray(sel.transpose(0, 2, 1))
    idx8 = np.arange(NT, dtype=np.float32)
    iota_tp1 = (np.arange(P, dtype=np.float32)[:, None]
                + 128.0 * idx8[None, :] + 1.0).astype(np.float32)
    iota_ec = np.broadcast_to(
        np.arange(NE, dtype=np.float32)[None, :] * float(C), (P, NE)).copy()

    shared = {
        "inwt": in_wt,
        "r_hi": R_hi,
        "r_lo": R_lo,
        "in_b": np.ascontiguousarray(in_b, dtype=np.float32),
        "out_b": np.ascontiguousarray(out_b, dtype=np.float32),
        "wio": _bf16(wio),
        "out_wt": _bf16(np.asarray(out_w, np.float32).T),
        "l_intra": l_intra,
        "lstrict64": lstrict64,
        "seltot": sel,
        "blkbc": blkbc,
        "iota_tp1": iota_tp1,
        "iota_ec": iota_ec,
    }
    in_maps = []
    for c in range(NCORES):
        xt = np.ascontiguousarray(x[c * TLOC:(c + 1) * TLOC].T)      # (D, TLOC)
        xt_hi = _trunc22(xt)
        xt_lo = _trunc22(xt - xt_hi)
        in_maps.append({"xt_hi": xt_hi, "xt_lo": xt_lo, **shared})
    return in_maps


def kernel(**inputs) -> np.ndarray:
    global _CACHED
    if _CACHED is None:
        _CACHED = build_program()
    nc = _CACHED
    in_maps = _prep(
        np.asarray(inputs["x"]), np.asarray(inputs["in_w"]),
        np.asarray(inputs["in_b"]), np.asarray(inputs["router_w"]),
        np.asarray(inputs["w_in"]), np.asarray(inputs["w_out"]),
        np.asarray(inputs["out_w"]), np.asarray(inputs["out_b"]))
    global LAST_RESULT
    res = run_bass_kernel_spmd(
        nc, in_maps, core_ids=list(range(NCORES)), trace=TRACE)
    LAST_RESULT = res
    # out_t columns are in sigma order: sigma(t) = a*128 + k*16 + b for
    # t = 128k + 8b + a
    t = np.arange(TLOC)
    sigma = (t % 8) * 128 + (t // 128) * 16 + ((t % 128) // 8)
    return np.concatenate(
        [np.ascontiguousarray(res.results[c]["out_t"].T[sigma])
         for c in range(NCORES)],
        axis=0)


# revision 44
# speedup vs baseline: 1.5275x; 1.5275x over previous
"""
MultiHeadLatentMoE layer as a Bass/Tile kernel for 8 Trainium2 NeuronCores.

Problem (T=8192, D=1024, NH=8 heads, DH=128, NE=8 experts/head, top-2, DHID=512):
    h      = (x @ in_w.T + in_b).reshape(T, NH, DH)
    logits = einsum('tnd,ned->tne', h, router_w)            (fp32)
    gate   = scatter(softmax(top2(logits)))                  (T, NH, NE)
    hid    = gelu(einsum('tnd,nefd->tnef', h, w_in))         (exact erf gelu)
    ye     = einsum('tnef,nefd->tned', hid, w_out)
    y      = einsum('tne,tned->tnd', gate, ye)
    out    = y.reshape(T, NH*DH) @ out_w.T + out_b

Sharding: pure data-parallel over tokens (1024 tokens/core, all heads+experts
local) -> zero collectives.  Per-core output shard is (D, T_loc) transposed;
host concatenates.

vs the previous dense version: the in-projection runs ONE fp32r pass (not a
3-term hi/lo split) because routing no longer uses h — logits come from
x @ R with R = in_w^T-blocks @ router_w folded on the host in fp64, computed
as a 3-term fp22 hi/lo split (verified 0/65536 top-2 flips on the reference
input).  Expert FFNs and the gate multiply run in bf16 (halves weight DMA
and doubles DVE throughput); expert matmuls accumulate in fp32 PSUM.
"""

import sys

for _p in ("/opt/trn_rl_repo", "/root/.axon_site/_ro/trn_rl_repo"):
    if _p not in sys.path:
        sys.path.append(_p)

import numpy as np
import ml_dtypes

import concourse.bass as bass
import concourse.mybir as mybir
import concourse.tile as tile
from concourse import bacc
from concourse.bass_utils import run_bass_kernel_spmd
from concourse.masks import make_identity

T, D, NH, DH, NE, TOPK, DHID = 8192, 1024, 8, 128, 8, 2, 512
NCORES = 8
TLOC = T // NCORES          # 1024 tokens per core
P = 128
KT = D // P                 # 8 contraction k-tiles for D=1024
TT = TLOC // 512            # 2 moving tiles of 512 tokens
NT = TLOC // P              # 8 token tiles of 128 (router/gate)
FT = DHID // P              # 4 f-tiles per expert
F32 = mybir.dt.float32
F32R = mybir.dt.float32r
BF16 = mybir.dt.bfloat16

_CACHED = None
TRACE = False          # set True (e.g. from test.py) to neuron-profile the run
LAST_RESULT = None     # BassKernelResults of the most recent kernel() call


def build_program():
    nc = bacc.Bacc()

    xt_hi = nc.dram_tensor("xt_hi", [D, TLOC], F32R, kind="ExternalInput")
    xt_lo = nc.dram_tensor("xt_lo", [D, TLOC], F32R, kind="ExternalInput")
    inwt = nc.dram_tensor("inwt", [D, D], F32R, kind="ExternalInput")
    r_hi = nc.dram_tensor("r_hi", [D, NH * NE], F32R, kind="ExternalInput")
    r_lo = nc.dram_tensor("r_lo", [D, NH * NE], F32R, kind="ExternalInput")
    w_int = nc.dram_tensor("w_int", [NH, NE, DH, DHID], BF16, kind="ExternalInput")
    w_outt = nc.dram_tensor("w_outt", [NH, NE, DHID, DH], BF16, kind="ExternalInput")
    out_wt = nc.dram_tensor("out_wt", [D, D], F32R, kind="ExternalInput")
    in_b = nc.dram_tensor("in_b", [D], F32, kind="ExternalInput")
    out_b = nc.dram_tensor("out_b", [D], F32, kind="ExternalInput")
    gate_dram = nc.dram_tensor("gate_dram", [NE, NH, TLOC], BF16)
    out_t = nc.dram_tensor("out_t", [D, TLOC], F32, kind="ExternalOutput")

    Act = mybir.ActivationFunctionType
    Alu = mybir.AluOpType

    with tile.TileContext(nc) as tc:
        with (
            tc.tile_pool(name="persist", bufs=1) as persist,
            tc.tile_pool(name="work", bufs=2) as work,
        ):
            ident = persist.tile([P, P], F32, tag="ident")
            make_identity(nc, ident)
            h_bf = persist.tile([P, NH, TLOC], BF16, tag="h_bf")  # experts input
            inb_sb = persist.tile([P, NH], F32, tag="inb")
            outb_sb = persist.tile([P, KT], F32, tag="outb")
            nc.sync.dma_start(inb_sb[:], in_b[:].rearrange("(n p) -> p n", p=P))
            nc.sync.dma_start(outb_sb[:], out_b[:].rearrange("(m p) -> p m", p=P))

            # ======= Phase 1: in-projection (single fp32r pass) + router =====
            with tc.tile_pool(name="xpool", bufs=1) as xpool, \
                 tc.tile_pool(name="psum", bufs=1, space="PSUM") as psum:
                x_hi = xpool.tile([P, KT, TLOC], F32R, tag="x_hi")
                x_lo = xpool.tile([P, KT, TLOC], F32R, tag="x_lo")
                inwt_sb = xpool.tile([P, KT, D], F32R, tag="inwt")
                rhi_sb = xpool.tile([P, KT, NH * NE], F32R, tag="rhi")
                rlo_sb = xpool.tile([P, KT, NH * NE], F32R, tag="rlo")
                for kt in range(KT):
                    sl = slice(kt * P, (kt + 1) * P)
                    nc.sync.dma_start(x_hi[:, kt, :], xt_hi[sl, :])
                    nc.sync.dma_start(x_lo[:, kt, :], xt_lo[sl, :])
                    nc.sync.dma_start(inwt_sb[:, kt, :], inwt[sl, :])
                nc.sync.dma_start(
                    rhi_sb[:], r_hi[:].rearrange("(kt p) f -> p kt f", p=P))
                nc.sync.dma_start(
                    rlo_sb[:], r_lo[:].rearrange("(kt p) f -> p kt f", p=P))

                # router logits^T = x @ R (3-term fp22 split; exact routing)
                lg_sb = work.tile([64, TLOC], F32, tag="lgT", bufs=1)
                for tt in range(TT):
                    tsl = slice(tt * 512, (tt + 1) * 512)
                    lgt_ps = psum.tile([64, 512], F32, tag="lgt", bufs=2)
                    terms = [(rhi_sb, x_hi), (rhi_sb, x_lo), (rlo_sb, x_hi)]
                    for i, (rv, xv) in enumerate(terms):
                        for kt in range(KT):
                            nc.tensor.matmul(
                                lgt_ps[:],
                                lhsT=rv[:, kt, :],
                                rhs=xv[:, kt, tsl],
                                start=(i == 0 and kt == 0),
                                stop=(i == 2 and kt == KT - 1),
                            )
                    nc.scalar.copy(lg_sb[:, tsl], lgt_ps[:])

                # h = x_hi @ inwt + in_b  (single pass, bf16 out for experts)
                for n in range(NH):
                    csl = slice(n * DH, (n + 1) * DH)
                    for tt in range(TT):
                        tsl = slice(tt * 512, (tt + 1) * 512)
                        h_ps = psum.tile([P, 512], F32, tag="hps", bufs=2)
                        for kt in range(KT):
                            nc.tensor.matmul(
                                h_ps[:],
                                lhsT=inwt_sb[:, kt, csl],
                                rhs=x_hi[:, kt, tsl],
                                start=(kt == 0),
                                stop=(kt == KT - 1),
                            )
                        nc.scalar.activation(
                            h_bf[:, n, tsl], h_ps[:], Act.Identity,
                            bias=inb_sb[:, n:n + 1])

                # ======= Phase 2: top-2 gate from logits^T ===================
                gate_t8 = persist.tile([NE, NH, TLOC], BF16, tag="gate_t8")
                for tk in range(NT):
                    ksl = slice(tk * P, (tk + 1) * P)
                    lg_ps = psum.tile([P, 64], F32, tag="lgtp", bufs=2)
                    nc.tensor.transpose(lg_ps[:], lg_sb[:, ksl], ident[:64, :64])
                    lg = work.tile([P, NH, NE], F32, tag="lg")
                    nc.vector.tensor_copy(
                        lg[:].rearrange("p n e -> p (n e)"), lg_ps[:])
                    m1 = work.tile([P, NH], F32, tag="m1")
                    nc.vector.tensor_reduce(m1[:], lg[:], mybir.AxisListType.X, Alu.max)
                    eq1 = work.tile([P, NH, NE], F32, tag="eq1")
                    nc.vector.tensor_tensor(
                        eq1[:], lg[:], m1[:, :, None].to_broadcast([P, NH, NE]),
                        Alu.is_equal)
                    msk = work.tile([P, NH, NE], F32, tag="msk")
                    nc.vector.scalar_tensor_tensor(
                        msk[:], eq1[:], -1e30, lg[:], Alu.mult, Alu.add)
                    m2 = work.tile([P, NH], F32, tag="m2")
                    nc.vector.tensor_reduce(m2[:], msk[:], mybir.AxisListType.X, Alu.max)
                    eq2 = work.tile([P, NH, NE], F32, tag="eq2")
                    nc.vector.tensor_tensor(
                        eq2[:], lg[:], m2[:, :, None].to_broadcast([P, NH, NE]),
                        Alu.is_equal)
                    dm = work.tile([P, NH], F32, tag="dm")
                    nc.vector.tensor_sub(dm[:], m2[:], m1[:])
                    w2 = work.tile([P, NH], F32, tag="w2")
                    nc.scalar.activation(w2[:], dm[:], Act.Sigmoid)
                    w1 = work.tile([P, NH], F32, tag="w1")
                    nc.vector.tensor_scalar(w1[:], w2[:], -1.0, 1.0, Alu.mult, Alu.add)
                    g1 = work.tile([P, NH, NE], F32, tag="g1")
                    nc.vector.tensor_tensor(
                        g1[:], eq1[:], w1[:, :, None].to_broadcast([P, NH, NE]), Alu.mult)
                    g2 = work.tile([P, NH, NE], F32, tag="g2")
                    nc.vector.tensor_tensor(
                        g2[:], eq2[:], w2[:, :, None].to_broadcast([P, NH, NE]), Alu.mult)
                    gk = work.tile([P, NH * NE], F32, tag="gk")
                    nc.vector.tensor_tensor(
                        gk[:].rearrange("p (n e) -> p n e", n=NH), g1[:], g2[:], Alu.add)
                    for n in range(NH):
                        tp_ps = psum.tile([NE, P], F32, tag="misc", bufs=2)
                        nc.tensor.transpose(
                            tp_ps[:], gk[:, n * NE:(n + 1) * NE], ident[:])
                        nc.vector.tensor_copy(gate_t8[:, n, ksl], tp_ps[:])

                nc.sync.dma_start(gate_dram[:], gate_t8[:])

            # ======= Phase 3: experts (dense, bf16) ==========================
            y_sb = persist.tile([P, NH, TLOC], F32R, tag="y")
            with tc.tile_pool(name="epool", bufs=3) as epool, \
                 tc.tile_pool(name="gpool", bufs=3) as gpool, \
                 tc.tile_pool(name="psum", bufs=1, space="PSUM") as psum:
                for n in range(NH):
                    y_ps = psum.tile([P, TT, 512], F32, tag="y", bufs=1)
                    for e in range(NE):
                        wi = epool.tile([P, DHID], BF16, tag="wi")
                        wo = epool.tile([P, FT, DH], BF16, tag="wo")
                        nc.sync.dma_start(wi[:], w_int[n, e])
                        nc.sync.dma_start(
                            wo[:], w_outt[n, e].rearrange("(kt p) d -> p kt d", p=P))
                        gbc_sb = gpool.tile([P, TLOC], BF16, tag="gbc_sb")
                        nc.sync.dma_start(
                            gbc_sb[:],
                            gate_dram[e, n][None, :].to_broadcast([P, TLOC]))
                        for tt in range(TT):
                            tsl = slice(tt * 512, (tt + 1) * 512)
                            for hf in range(2):
                                hid_ps = psum.tile(
                                    [P, 2, 512], F32, tag=f"hid{hf}", bufs=1)
                                for fi in range(2):
                                    f = hf * 2 + fi
                                    nc.tensor.matmul(
                                        hid_ps[:, fi, :],
                                        lhsT=wi[:, f * P:(f + 1) * P],
                                        rhs=h_bf[:, n, tsl],
                                        start=True, stop=True,
                                    )
                                hidg = gpool.tile([P, 2, 512], BF16, tag="hidg")
                                nc.scalar.activation(hidg[:], hid_ps[:], Act.Gelu)
                                hidg_r = gpool.tile([P, 2, 512], BF16, tag="hidg_r")
                                nc.vector.tensor_tensor(
                                    hidg_r[:], hidg[:],
                                    gbc_sb[:, tsl][:, None, :].to_broadcast(
                                        [P, 2, 512]),
                                    Alu.mult)
                                for kt in range(2):
                                    nc.tensor.matmul(
                                        y_ps[:, tt, :],
                                        lhsT=wo[:, hf * 2 + kt, :],
                                        rhs=hidg_r[:, kt, :],
                                        start=(e == 0 and hf == 0 and kt == 0),
                                        stop=(e == NE - 1 and hf == 1 and kt == 1),
                                    )
                    nc.vector.tensor_copy(
                        y_sb[:, n, :], y_ps[:].rearrange("p a b -> p (a b)"))

            # ======= Phase 4: out-projection (fp32r) =========================
            with tc.tile_pool(name="opool", bufs=2) as opool, \
                 tc.tile_pool(name="psum", bufs=1, space="PSUM") as psum:
                for m in range(KT):
                    ow = opool.tile([P, KT, P], F32R, tag="ow")
                    nc.sync.dma_start(
                        ow[:],
                        out_wt[:, m * P:(m + 1) * P].rearrange(
                            "(kt p) d -> p kt d", p=P))
                    o_sb = opool.tile([P, TLOC], F32, tag="osb")
                    for tt in range(TT):
                        tsl = slice(tt * 512, (tt + 1) * 512)
                        o_ps = psum.tile([P, 512], F32, tag="misc", bufs=2)
                        for kt in range(KT):
                            nc.tensor.matmul(
                                o_ps[:],
                                lhsT=ow[:, kt, :],
                                rhs=y_sb[:, kt, tsl],
                                start=(kt == 0),
                                stop=(kt == KT - 1),
                            )
                        nc.scalar.activation(
                            o_sb[:, tsl], o_ps[:], Act.Identity,
                            bias=outb_sb[:, m:m + 1])
                    nc.sync.dma_start(out_t[m * P:(m + 1) * P, :], o_sb[:])

    nc.compile()
    return nc


def _trunc22(a):
    """FP32 -> FP22 truncation (the read path of float32r matmuls)."""
    return (np.ascontiguousarray(a, np.float32).view(np.uint32)
            & np.uint32(0xFFFFE000)).view(np.float32)


def _bf16(a):
    return np.ascontiguousarray(a, np.float32).astype(ml_dtypes.bfloat16)


def _prep(x, in_w, in_b, router_w, w_in, w_out, out_w, out_b):
    """Host-side lossless layout prep; returns per-core in_maps."""
    x = np.ascontiguousarray(x, dtype=np.float32)
    in_wt = np.ascontiguousarray(in_w.T, dtype=np.float32)           # (D, D)
    R = np.einsum(
        'dnh,neh->dne',
        in_wt.astype(np.float64).reshape(D, NH, DH),
        np.asarray(router_w, np.float64)).astype(np.float32).reshape(D, NH * NE)
    R_hi = _trunc22(R)
    R_lo = _trunc22(R - R_hi)
    rb = np.einsum('nh,neh->ne', np.asarray(in_b, np.float64).reshape(NH, DH),
                   np.asarray(router_w, np.float64))
    assert np.abs(rb).max() < 1e-30, "nonzero in_b needs router bias support"
    shared = {
        "inwt": in_wt,
        "r_hi": R_hi,
        "r_lo": R_lo,
        "w_int": _bf16(np.asarray(w_in, np.float32).transpose(0, 1, 3, 2)),
        "w_outt": _bf16(w_out),
        "out_wt": np.ascontiguousarray(out_w.T, dtype=np.float32),
        "in_b": np.ascontiguousarray(in_b, dtype=np.float32),
        "out_b": np.ascontiguousarray(out_b, dtype=np.float32),
    }
    in_maps = []
    for c in range(NCORES):
        xt = np.ascontiguousarray(x[c * TLOC:(c + 1) * TLOC].T)      # (D, TLOC)
        xt_hi = _trunc22(xt)
        xt_lo = _trunc22(xt - xt_hi)
        in_maps.append({"xt_hi": xt_hi, "xt_lo": xt_lo, **shared})
    return in_maps


def kernel(**inputs) -> np.ndarray:
    global _CACHED
    if _CACHED is None:
        _CACHED = build_program()
    nc = _CACHED
    in_maps = _prep(
        np.asarray(inputs["x"]), np.asarray(inputs["in_w"]),
        np.asarray(inputs["in_b"]), np.asarray(inputs["router_w"]),
        np.asarray(inputs["w_in"]), np.asarray(inputs["w_out"]),
        np.asarray(inputs["out_w"]), np.asarray(inputs["out_b"]))
    global LAST_RESULT
    res = run_bass_kernel_spmd(
        nc, in_maps, core_ids=list(range(NCORES)), trace=TRACE)
    LAST_RESULT = res
    return np.concatenate(
        [np.ascontiguousarray(res.results[c]["out_t"].T) for c in range(NCORES)],
        axis=0)


# revision 45
# speedup vs baseline: 1.5635x; 1.0236x over previous
"""
MultiHeadLatentMoE layer as a Bass/Tile kernel for 8 Trainium2 NeuronCores.

Problem (T=8192, D=1024, NH=8 heads, DH=128, NE=8 experts/head, top-2, DHID=512):
    h      = (x @ in_w.T + in_b).reshape(T, NH, DH)
    logits = einsum('tnd,ned->tne', h, router_w)            (fp32)
    gate   = scatter(softmax(top2(logits)))                  (T, NH, NE)
    hid    = gelu(einsum('tnd,nefd->tnef', h, w_in))         (exact erf gelu)
    ye     = einsum('tnef,nefd->tned', hid, w_out)
    y      = einsum('tne,tned->tnd', gate, ye)
    out    = y.reshape(T, NH*DH) @ out_w.T + out_b

Sharding: pure data-parallel over tokens (1024 tokens/core, all heads+experts
local) -> zero collectives.  Per-core output shard is (D, T_loc) transposed;
host concatenates.

vs the previous dense version: the in-projection runs ONE fp32r pass (not a
3-term hi/lo split) because routing no longer uses h — logits come from
x @ R with R = in_w^T-blocks @ router_w folded on the host in fp64, computed
as a 3-term fp22 hi/lo split (verified 0/65536 top-2 flips on the reference
input).  Expert FFNs and the gate multiply run in bf16 (halves weight DMA
and doubles DVE throughput); expert matmuls accumulate in fp32 PSUM.
"""

import sys

for _p in ("/opt/trn_rl_repo", "/root/.axon_site/_ro/trn_rl_repo"):
    if _p not in sys.path:
        sys.path.append(_p)

import numpy as np
import ml_dtypes

import concourse.bass as bass
import concourse.mybir as mybir
import concourse.tile as tile
from concourse import bacc
from concourse.bass_utils import run_bass_kernel_spmd
from concourse.masks import make_identity

T, D, NH, DH, NE, TOPK, DHID = 8192, 1024, 8, 128, 8, 2, 512
NCORES = 8
TLOC = T // NCORES          # 1024 tokens per core
P = 128
KT = D // P                 # 8 contraction k-tiles for D=1024
TT = TLOC // 512            # 2 moving tiles of 512 tokens
NT = TLOC // P              # 8 token tiles of 128 (router/gate)
FT = DHID // P              # 4 f-tiles per expert
F32 = mybir.dt.float32
F32R = mybir.dt.float32r
BF16 = mybir.dt.bfloat16

_CACHED = None
TRACE = False          # set True (e.g. from test.py) to neuron-profile the run
LAST_RESULT = None     # BassKernelResults of the most recent kernel() call


def build_program():
    nc = bacc.Bacc()

    xt_hi = nc.dram_tensor("xt_hi", [D, TLOC], F32R, kind="ExternalInput")
    xt_lo = nc.dram_tensor("xt_lo", [D, TLOC], F32R, kind="ExternalInput")
    inwt = nc.dram_tensor("inwt", [D, D], F32R, kind="ExternalInput")
    r_hi = nc.dram_tensor("r_hi", [D, NH * NE], F32R, kind="ExternalInput")
    r_lo = nc.dram_tensor("r_lo", [D, NH * NE], F32R, kind="ExternalInput")
    w_int = nc.dram_tensor("w_int", [NH, NE, DH, DHID], BF16, kind="ExternalInput")
    w_outt = nc.dram_tensor("w_outt", [NH, NE, DHID, DH], BF16, kind="ExternalInput")
    out_wt = nc.dram_tensor("out_wt", [D, D], F32R, kind="ExternalInput")
    in_b = nc.dram_tensor("in_b", [D], F32, kind="ExternalInput")
    out_b = nc.dram_tensor("out_b", [D], F32, kind="ExternalInput")
    gate_dram = nc.dram_tensor("gate_dram", [NE, NH, TLOC], BF16)
    out_t = nc.dram_tensor("out_t", [D, TLOC], F32, kind="ExternalOutput")

    Act = mybir.ActivationFunctionType
    Alu = mybir.AluOpType

    with tile.TileContext(nc) as tc:
        with (
            tc.tile_pool(name="persist", bufs=1) as persist,
            tc.tile_pool(name="work", bufs=2) as work,
        ):
            ident = persist.tile([P, P], F32, tag="ident")
            make_identity(nc, ident)
            h_bf = persist.tile([P, NH, TLOC], BF16, tag="h_bf")  # experts input
            inb_sb = persist.tile([P, NH], F32, tag="inb")
            outb_sb = persist.tile([P, KT], F32, tag="outb")
            nc.sync.dma_start(inb_sb[:], in_b[:].rearrange("(n p) -> p n", p=P))
            nc.sync.dma_start(outb_sb[:], out_b[:].rearrange("(m p) -> p m", p=P))

            # ======= Phase 1: in-projection (single fp32r pass) + router =====
            with tc.tile_pool(name="xpool", bufs=1) as xpool, \
                 tc.tile_pool(name="psum", bufs=1, space="PSUM") as psum:
                x_hi = xpool.tile([P, KT, TLOC], F32R, tag="x_hi")
                x_lo = xpool.tile([P, KT, TLOC], F32R, tag="x_lo")
                inwt_sb = xpool.tile([P, KT, D], F32R, tag="inwt")
                rhi_sb = xpool.tile([P, KT, NH * NE], F32R, tag="rhi")
                rlo_sb = xpool.tile([P, KT, NH * NE], F32R, tag="rlo")
                for kt in range(KT):
                    sl = slice(kt * P, (kt + 1) * P)
                    nc.sync.dma_start(x_hi[:, kt, :], xt_hi[sl, :])
                    nc.sync.dma_start(x_lo[:, kt, :], xt_lo[sl, :])
                    nc.sync.dma_start(inwt_sb[:, kt, :], inwt[sl, :])
                nc.sync.dma_start(
                    rhi_sb[:], r_hi[:].rearrange("(kt p) f -> p kt f", p=P))
                nc.sync.dma_start(
                    rlo_sb[:], r_lo[:].rearrange("(kt p) f -> p kt f", p=P))

                # router logits^T = x @ R (3-term fp22 split; exact routing)
                lg_sb = work.tile([64, TLOC], F32, tag="lgT", bufs=1)
                for tt in range(TT):
                    tsl = slice(tt * 512, (tt + 1) * 512)
                    lgt_ps = psum.tile([64, 512], F32, tag="lgt", bufs=2)
                    terms = [(rhi_sb, x_hi), (rhi_sb, x_lo), (rlo_sb, x_hi)]
                    for i, (rv, xv) in enumerate(terms):
                        for kt in range(KT):
                            nc.tensor.matmul(
                                lgt_ps[:],
                                lhsT=rv[:, kt, :],
                                rhs=xv[:, kt, tsl],
                                start=(i == 0 and kt == 0),
                                stop=(i == 2 and kt == KT - 1),
                            )
                    nc.scalar.copy(lg_sb[:, tsl], lgt_ps[:])

                # h = x_hi @ inwt + in_b  (single pass, bf16 out for experts)
                for n in range(NH):
                    csl = slice(n * DH, (n + 1) * DH)
                    for tt in range(TT):
                        tsl = slice(tt * 512, (tt + 1) * 512)
                        h_ps = psum.tile([P, 512], F32, tag="hps", bufs=2)
                        for kt in range(KT):
                            nc.tensor.matmul(
                                h_ps[:],
                                lhsT=inwt_sb[:, kt, csl],
                                rhs=x_hi[:, kt, tsl],
                                start=(kt == 0),
                                stop=(kt == KT - 1),
                            )
                        nc.scalar.activation(
                            h_bf[:, n, tsl], h_ps[:], Act.Identity,
                            bias=inb_sb[:, n:n + 1])

                # ======= Phase 2: top-2 gate from logits^T ===================
                gate_t8 = persist.tile([NE, NH, TLOC], BF16, tag="gate_t8")
                for tk in range(NT):
                    ksl = slice(tk * P, (tk + 1) * P)
                    lg_ps = psum.tile([P, 64], F32, tag="lgtp", bufs=2)
                    nc.tensor.transpose(lg_ps[:], lg_sb[:, ksl], ident[:64, :64])
                    lg = work.tile([P, NH, NE], F32, tag="lg")
                    nc.vector.tensor_copy(
                        lg[:].rearrange("p n e -> p (n e)"), lg_ps[:])
                    m1 = work.tile([P, NH], F32, tag="m1")
                    nc.vector.tensor_reduce(m1[:], lg[:], mybir.AxisListType.X, Alu.max)
                    eq1 = work.tile([P, NH, NE], F32, tag="eq1")
                    nc.vector.tensor_tensor(
                        eq1[:], lg[:], m1[:, :, None].to_broadcast([P, NH, NE]),
                        Alu.is_equal)
                    msk = work.tile([P, NH, NE], F32, tag="msk")
                    nc.vector.scalar_tensor_tensor(
                        msk[:], eq1[:], -1e30, lg[:], Alu.mult, Alu.add)
                    m2 = work.tile([P, NH], F32, tag="m2")
                    nc.vector.tensor_reduce(m2[:], msk[:], mybir.AxisListType.X, Alu.max)
                    eq2 = work.tile([P, NH, NE], F32, tag="eq2")
                    nc.vector.tensor_tensor(
                        eq2[:], lg[:], m2[:, :, None].to_broadcast([P, NH, NE]),
                        Alu.is_equal)
                    dm = work.tile([P, NH], F32, tag="dm")
                    nc.vector.tensor_sub(dm[:], m2[:], m1[:])
                    w2 = work.tile([P, NH], F32, tag="w2")
                    nc.scalar.activation(w2[:], dm[:], Act.Sigmoid)
                    w1 = work.tile([P, NH], F32, tag="w1")
                    nc.vector.tensor_scalar(w1[:], w2[:], -1.0, 1.0, Alu.mult, Alu.add)
                    g1 = work.tile([P, NH, NE], F32, tag="g1")
                    nc.vector.tensor_tensor(
                        g1[:], eq1[:], w1[:, :, None].to_broadcast([P, NH, NE]), Alu.mult)
                    g2 = work.tile([P, NH, NE], F32, tag="g2")
                    nc.vector.tensor_tensor(
                        g2[:], eq2[:], w2[:, :, None].to_broadcast([P, NH, NE]), Alu.mult)
                    gk = work.tile([P, NH * NE], F32, tag="gk")
                    nc.vector.tensor_tensor(
                        gk[:].rearrange("p (n e) -> p n e", n=NH), g1[:], g2[:], Alu.add)
                    for n in range(NH):
                        tp_ps = psum.tile([NE, P], F32, tag="misc", bufs=2)
                        nc.tensor.transpose(
                            tp_ps[:], gk[:, n * NE:(n + 1) * NE], ident[:])
                        nc.vector.tensor_copy(gate_t8[:, n, ksl], tp_ps[:])

                nc.sync.dma_start(gate_dram[:], gate_t8[:])

            # ======= Phase 3: experts (dense, bf16) ==========================
            y_sb = persist.tile([P, NH, TLOC], F32R, tag="y")
            with tc.tile_pool(name="epool", bufs=3) as epool, \
                 tc.tile_pool(name="gpool", bufs=3) as gpool, \
                 tc.tile_pool(name="psum", bufs=1, space="PSUM") as psum:
                for n in range(NH):
                    y_ps = psum.tile([P, TT, 512], F32, tag="y", bufs=1)
                    for e in range(NE):
                        wi = epool.tile([P, DHID], BF16, tag="wi")
                        wo = epool.tile([P, FT, DH], BF16, tag="wo")
                        nc.sync.dma_start(wi[:], w_int[n, e])
                        nc.sync.dma_start(
                            wo[:], w_outt[n, e].rearrange("(kt p) d -> p kt d", p=P))
                        gbc_sb = gpool.tile([P, TLOC], BF16, tag="gbc_sb")
                        nc.sync.dma_start(
                            gbc_sb[:],
                            gate_dram[e, n][None, :].to_broadcast([P, TLOC]))
                        for tt in range(TT):
                            tsl = slice(tt * 512, (tt + 1) * 512)
                            for hf in range(2):
                                # 3-deep rotation: PE fills unit i+2 while ACT
                                # gelus i+1 and DVE scales i (chain ~1.5us vs
                                # PE ~1.1us per unit)
                                hid_ps = psum.tile(
                                    [P, 2, 512], F32, tag="hid", bufs=3)
                                for fi in range(2):
                                    f = hf * 2 + fi
                                    nc.tensor.matmul(
                                        hid_ps[:, fi, :],
                                        lhsT=wi[:, f * P:(f + 1) * P],
                                        rhs=h_bf[:, n, tsl],
                                        start=True, stop=True,
                                    )
                                hidg = gpool.tile([P, 2, 512], BF16, tag="hidg")
                                nc.scalar.activation(hidg[:], hid_ps[:], Act.Gelu)
                                hidg_r = gpool.tile([P, 2, 512], BF16, tag="hidg_r")
                                nc.vector.tensor_tensor(
                                    hidg_r[:], hidg[:],
                                    gbc_sb[:, tsl][:, None, :].to_broadcast(
                                        [P, 2, 512]),
                                    Alu.mult)
                                for kt in range(2):
                                    nc.tensor.matmul(
                                        y_ps[:, tt, :],
                                        lhsT=wo[:, hf * 2 + kt, :],
                                        rhs=hidg_r[:, kt, :],
                                        start=(e == 0 and hf == 0 and kt == 0),
                                        stop=(e == NE - 1 and hf == 1 and kt == 1),
                                    )
                    nc.vector.tensor_copy(
                        y_sb[:, n, :], y_ps[:].rearrange("p a b -> p (a b)"))

            # ======= Phase 4: out-projection (fp32r) =========================
            with tc.tile_pool(name="opool", bufs=2) as opool, \
                 tc.tile_pool(name="psum", bufs=1, space="PSUM") as psum:
                for m in range(KT):
                    ow = opool.tile([P, KT, P], F32R, tag="ow")
                    nc.sync.dma_start(
                        ow[:],
                        out_wt[:, m * P:(m + 1) * P].rearrange(
                            "(kt p) d -> p kt d", p=P))
                    o_sb = opool.tile([P, TLOC], F32, tag="osb")
                    for tt in range(TT):
                        tsl = slice(tt * 512, (tt + 1) * 512)
                        o_ps = psum.tile([P, 512], F32, tag="misc", bufs=2)
                        for kt in range(KT):
                            nc.tensor.matmul(
                                o_ps[:],
                                lhsT=ow[:, kt, :],
                                rhs=y_sb[:, kt, tsl],
                                start=(kt == 0),
                                stop=(kt == KT - 1),
                            )
                        nc.scalar.activation(
                            o_sb[:, tsl], o_ps[:], Act.Identity,
                            bias=outb_sb[:, m:m + 1])
                    nc.sync.dma_start(out_t[m * P:(m + 1) * P, :], o_sb[:])

    nc.compile()
    return nc


def _trunc22(a):
    """FP32 -> FP22 truncation (the read path of float32r matmuls)."""
    return (np.ascontiguousarray(a, np.float32).view(np.uint32)
            & np.uint32(0xFFFFE000)).view(np.float32)


def _bf16(a):
    return np.ascontiguousarray(a, np.float32).astype(ml_dtypes.bfloat16)


def _prep(x, in_w, in_b, router_w, w_in, w_out, out_w, out_b):
    """Host-side lossless layout prep; returns per-core in_maps."""
    x = np.ascontiguousarray(x, dtype=np.float32)
    in_wt = np.ascontiguousarray(in_w.T, dtype=np.float32)           # (D, D)
    R = np.einsum(
        'dnh,neh->dne',
        in_wt.astype(np.float64).reshape(D, NH, DH),
        np.asarray(router_w, np.float64)).astype(np.float32).reshape(D, NH * NE)
    R_hi = _trunc22(R)
    R_lo = _trunc22(R - R_hi)
    rb = np.einsum('nh,neh->ne', np.asarray(in_b, np.float64).reshape(NH, DH),
                   np.asarray(router_w, np.float64))
    assert np.abs(rb).max() < 1e-30, "nonzero in_b needs router bias support"
    shared = {
        "inwt": in_wt,
        "r_hi": R_hi,
        "r_lo": R_lo,
        "w_int": _bf16(np.asarray(w_in, np.float32).transpose(0, 1, 3, 2)),
        "w_outt": _bf16(w_out),
        "out_wt": np.ascontiguousarray(out_w.T, dtype=np.float32),
        "in_b": np.ascontiguousarray(in_b, dtype=np.float32),
        "out_b": np.ascontiguousarray(out_b, dtype=np.float32),
    }
    in_maps = []
    for c in range(NCORES):
        xt = np.ascontiguousarray(x[c * TLOC:(c + 1) * TLOC].T)      # (D, TLOC)
        xt_hi = _trunc22(xt)
        xt_lo = _trunc22(xt - xt_hi)
        in_maps.append({"xt_hi": xt_hi, "xt_lo": xt_lo, **shared})
    return in_maps


def kernel(**inputs) -> np.ndarray:
    global _CACHED
    if _CACHED is None:
        _CACHED = build_program()
    nc = _CACHED
    in_maps = _prep(
        np.asarray(inputs["x"]), np.asarray(inputs["in_w"]),
        np.asarray(inputs["in_b"]), np.asarray(inputs["router_w"]),
        np.asarray(inputs["w_in"]), np.asarray(inputs["w_out"]),
        np.asarray(inputs["out_w"]), np.asarray(inputs["out_b"]))
    global LAST_RESULT
    res = run_bass_kernel_spmd(
        nc, in_maps, core_ids=list(range(NCORES)), trace=TRACE)
    LAST_RESULT = res
    return np.concatenate(
        [np.ascontiguousarray(res.results[c]["out_t"].T) for c in range(NCORES)],
        axis=0)
